# revision 1
# baseline (speedup 1.0000x reference)
"""Trainium2 Bass kernel for nn_NeighbourLoss (NeighbourLoss with k=1, margin=0.1).

Strategy (8-core data parallel, rows sharded):
  Reference computation per row i over all j:
    dist[i,j] = sqrt(clip(sq_i + sq_j - 2 x_i.x_j, 1e-12))
    thresh_i  = min over positives (same class, j!=i) of dist + margin   [k=1 => min, no sort]
    cnt_i     = #{j in neg: dist < thresh_i},  sum_sel_i = sum of those dists
    loss      = mean over valid rows of (min_pos - mean_neg + margin)
  Host prologue: the positive set is tiny (8 partners/row) -> compute min_pos/thresh
  plus same-class correction terms exactly on the host (numpy, fp64).
  Device (per core, rows shard of 1024): one pass over its [1024, 8192] slice of
  the distance matrix:
    PSUM d2 via 5-chunk fp32r matmul (x chunks scaled by -2 on the moving side,
    plus a K=2 ones x (sq_hi, sq_lo) chunk folding in sq_j; sq_i enters via the
    ACT bias); a 6th matmul adds BIG=2^40 on the diagonal (columns are rotated
    per-core so the diagonal block lands at the same device tile on every core,
    keeping the program SPMD-identical).
    ACT: dist = Sqrt(psum + sq_i) with accum -> per-row sum of dist.
    DVE: tensor_scalar is_lt(thresh_i) with accum -> per-row count below thresh.
    ACT: Relu(thresh_i - dist) with accum -> per-row sum of (thresh - dist)+ .
  Host epilogue: subtract the (exactly known) diagonal and same-class
  contributions, recover sum_sel = cnt*thresh - sum_relu, assemble the 4 outputs.
"""
import sys

sys.path.insert(0, "/opt/trn_rl_repo")

import numpy as np

N_ROWS = 8192
DIM = 512
N_CORES = 8
ROWS_PER_CORE = N_ROWS // N_CORES          # 1024
N_RT = ROWS_PER_CORE // 128                # 8 row tiles per core
N_CT = N_ROWS // 512                       # 16 col tiles (512 wide)
N_GROUPS = 4                               # col groups of 2048
CT_PER_G = N_CT // N_GROUPS                # 4
MARGIN = np.float32(0.1)
BIG = np.float32(2.0 ** 40)                # diag pusher; sqrt(2^40) = 2^20 exactly
BIG_SQRT = np.float64(2.0 ** 20)

_CACHE = {}


def _build_nc():
    import concourse.bass as bass  # noqa: F401
    from concourse import bacc, mybir
    import concourse.tile as tile

    f32 = mybir.dt.float32
    f32r = mybir.dt.float32r

    nc = bacc.Bacc("TRN2", debug=False)

    xm2T_in = nc.declare_dram_parameter("xm2T", [DIM, N_ROWS], f32r, isOutput=False)
    sqhl_in = nc.declare_dram_parameter("sqhl", [2, N_ROWS], f32r, isOutput=False)
    xTown_in = nc.declare_dram_parameter("xTown", [DIM, ROWS_PER_CORE], f32r, isOutput=False)
    ones2_in = nc.declare_dram_parameter("ones2", [2, ROWS_PER_CORE], f32r, isOutput=False)
    diagv_in = nc.declare_dram_parameter("diagv", [128, 4 * 512], f32r, isOutput=False)
    ident_in = nc.declare_dram_parameter("ident", [128, 128], f32r, isOutput=False)
    sqown_in = nc.declare_dram_parameter("sqown", [128, N_RT], f32, isOutput=False)
    throw_in = nc.declare_dram_parameter("throw", [128, N_RT], f32, isOutput=False)

    osum_out = nc.declare_dram_parameter("osum", [128, N_RT * N_CT], f32, isOutput=True)
    ocnt_out = nc.declare_dram_parameter("ocnt", [128, N_RT * N_GROUPS], f32, isOutput=True)
    orel_out = nc.declare_dram_parameter("orel", [128, N_RT * N_GROUPS], f32, isOutput=True)

    GW = 2048  # group width in columns

    with tile.TileContext(nc) as tc:
        with tc.tile_pool(name="const", bufs=1) as cpool, \
             tc.tile_pool(name="rhs", bufs=2) as rhspool, \
             tc.tile_pool(name="dist", bufs=3) as dpool, \
             tc.tile_pool(name="scr", bufs=2) as spool, \
             tc.tile_pool(name="scr2", bufs=2) as spool2, \
             tc.tile_pool(name="acc", bufs=1) as apool, \
             tc.tile_pool(name="ps", bufs=7, space="PSUM") as pspool:

            # ---- resident constants ----
            lhsT = []
            for kc in range(4):
                t = cpool.tile([128, ROWS_PER_CORE], f32r, tag=f"lhsT{kc}")
                nc.sync.dma_start(t[:], xTown_in[kc * 128:(kc + 1) * 128, :])
                lhsT.append(t)
            ones2_t = cpool.tile([2, ROWS_PER_CORE], f32r, tag="ones2")
            nc.sync.dma_start(ones2_t[:], ones2_in[:])
            diag_t = cpool.tile([128, 4 * 512], f32r, tag="diag")
            nc.sync.dma_start(diag_t[:], diagv_in[:])
            ident_t = cpool.tile([128, 128], f32r, tag="ident")
            nc.sync.dma_start(ident_t[:], ident_in[:])
            sqown_t = cpool.tile([128, N_RT], f32, tag="sqown")
            nc.sync.dma_start(sqown_t[:], sqown_in[:])
            throw_t = cpool.tile([128, N_RT], f32, tag="throw")
            nc.sync.dma_start(throw_t[:], throw_in[:])

            osum_t = apool.tile([128, N_RT * N_CT], f32, tag="osum")
            ocnt_t = apool.tile([128, N_RT * N_GROUPS], f32, tag="ocnt")
            orel_t = apool.tile([128, N_RT * N_GROUPS], f32, tag="orel")

            for g in range(N_GROUPS):
                # ---- stream this column group's moving operands ----
                rhs = []
                for kc in range(4):
                    t = rhspool.tile([128, GW], f32r, tag=f"rhs{kc}")
                    nc.sync.dma_start(t[:], xm2T_in[kc * 128:(kc + 1) * 128,
                                                    g * GW:(g + 1) * GW])
                    rhs.append(t)
                sqg_t = rhspool.tile([2, GW], f32r, tag="sqg")
                nc.sync.dma_start(sqg_t[:], sqhl_in[:, g * GW:(g + 1) * GW])

                for rt in range(N_RT):
                    dist_g = dpool.tile([128, GW], f32, tag="dist")
                    for ct in range(CT_PER_G):
                        dev_ct = g * CT_PER_G + ct
                        is_diag = dev_ct == rt // 4
                        ps = pspool.tile([128, 512], f32, tag="ps")
                        for kc in range(4):
                            nc.tensor.matmul(
                                ps[:],
                                lhsT[kc][:, rt * 128:(rt + 1) * 128],
                                rhs[kc][:, ct * 512:(ct + 1) * 512],
                                start=(kc == 0), stop=False)
                        nc.tensor.matmul(
                            ps[:],
                            ones2_t[:, rt * 128:(rt + 1) * 128],
                            sqg_t[:, ct * 512:(ct + 1) * 512],
                            start=False, stop=(not is_diag))
                        if is_diag:
                            nc.tensor.matmul(
                                ps[:], ident_t[:],
                                diag_t[:, (rt % 4) * 512:(rt % 4 + 1) * 512],
                                start=False, stop=True)
                        from concourse import mybir as _mb
                        nc.scalar.activation(
                            dist_g[:, ct * 512:(ct + 1) * 512], ps[:],
                            _mb.ActivationFunctionType.Sqrt,
                            bias=sqown_t[:, rt:rt + 1], scale=1.0,
                            accum_out=osum_t[:, rt * N_CT + dev_ct:rt * N_CT + dev_ct + 1])
                    from concourse import mybir as _mb
                    scr = spool.tile([128, GW], f32, tag="scr")
                    nc.vector.tensor_scalar(
                        scr[:], dist_g[:], throw_t[:, rt:rt + 1], None,
                        op0=_mb.AluOpType.is_lt, op1=_mb.AluOpType.add,
                        accum_out=ocnt_t[:, rt * N_GROUPS + g:rt * N_GROUPS + g + 1])
                    scr2 = spool2.tile([128, GW], f32, tag="scr2")
                    nc.scalar.activation(
                        scr2[:], dist_g[:], _mb.ActivationFunctionType.Relu,
                        bias=throw_t[:, rt:rt + 1], scale=-1.0,
                        accum_out=orel_t[:, rt * N_GROUPS + g:rt * N_GROUPS + g + 1])

            nc.sync.dma_start(osum_out[:], osum_t[:])
            nc.sync.dma_start(ocnt_out[:], ocnt_t[:])
            nc.sync.dma_start(orel_out[:], orel_t[:])

    nc.finalize()
    return nc


def _get_runner():
    """Compile once; return a function(in_maps) -> list of per-core out dicts."""
    if "runner" in _CACHE:
        return _CACHE["runner"]

    import jax
    import jax.numpy as jnp  # noqa: F401
    from jax.sharding import Mesh, PartitionSpec
    try:
        from jax.experimental.shard_map import shard_map
    except Exception:
        from jax.shard_map import shard_map  # newer jax
    from concourse import bass2jax, mybir

    nc = _build_nc()
    bass2jax.install_neuronx_cc_hook()

    in_names, out_names, out_avals = [], [], []
    partition_name = nc.partition_id_tensor.name if nc.partition_id_tensor else None
    for alloc in nc.m.functions[0].allocations:
        if not isinstance(alloc, mybir.MemoryLocationSet):
            continue
        name = alloc.memorylocations[0].name
        if alloc.kind == "ExternalInput":
            if name != partition_name:
                in_names.append(name)
        elif alloc.kind == "ExternalOutput":
            out_names.append(name)
            out_avals.append(jax.core.ShapedArray(tuple(alloc.tensor_shape),
                                                  mybir.dt.np(alloc.dtype)))
    n_params = len(in_names)
    zero_outs = [np.zeros(a.shape, a.dtype) for a in out_avals]
    all_names = in_names + out_names
    if partition_name is not None:
        all_names.append(partition_name)

    def _body(*args):
        operands = list(args)
        if partition_name is not None:
            operands.append(bass2jax.partition_id_tensor())
        outs = bass2jax._bass_exec_p.bind(
            *operands,
            out_avals=tuple(out_avals),
            in_names=tuple(all_names),
            out_names=tuple(out_names),
            lowering_input_output_aliases=(),
            sim_require_finite=True,
            sim_require_nnan=True,
            nc=nc,
        )
        return tuple(outs)

    devices = jax.devices()[:N_CORES]
    assert len(devices) == N_CORES, f"need {N_CORES} cores, have {len(jax.devices())}"
    mesh = Mesh(np.asarray(devices), ("core",))
    n_args = n_params + len(out_names)
    sharded = jax.jit(
        shard_map(_body, mesh=mesh,
                  in_specs=(PartitionSpec("core"),) * n_args,
                  out_specs=(PartitionSpec("core"),) * len(out_names),
                  check_rep=False),
        keep_unused=True,
    )

    def run(in_maps, device_arrays=None):
        if device_arrays is None:
            concat_in = [np.concatenate([in_maps[c][name] for c in range(N_CORES)], axis=0)
                         for name in in_names]
            concat_zero = [np.zeros((N_CORES * z.shape[0], *z.shape[1:]), z.dtype)
                           for z in zero_outs]
            device_arrays = concat_in + concat_zero
        out_arrs = sharded(*device_arrays)
        return [
            {name: np.asarray(out_arrs[i]).reshape(N_CORES, *out_avals[i].shape)[c]
             for i, name in enumerate(out_names)}
            for c in range(N_CORES)
        ]

    _CACHE["runner"] = (run, in_names, out_names, zero_outs, sharded)
    return _CACHE["runner"]


def _trunc_hi(v32):
    """Split fp32 into a coarse (10-bit mantissa) hi part that any reduced
    matmul format represents exactly, plus the fp32 residual."""
    u = v32.view(np.uint32) & np.uint32(0xFFFFE000)
    hi = u.view(np.float32)
    lo = (v32.astype(np.float64) - hi.astype(np.float64)).astype(np.float32)
    return hi, lo


def _host_prologue(x, targets):
    """Exact (fp64) handling of the tiny positive-pair set."""
    n = x.shape[0]
    x64 = x.astype(np.float64)
    sq64 = (x64 * x64).sum(axis=1)

    order = np.argsort(targets, kind="stable")
    t_sorted = targets[order]
    counts = np.bincount(targets)
    per = counts.max()
    assert (counts == per).all(), "kernel assumes balanced classes"
    groups = order.reshape(-1, per)                       # [C, per]
    partners = np.empty((n, per), np.int64)
    partners[groups] = np.broadcast_to(groups[:, None, :], (groups.shape[0], per, per))

    xg = x64[partners]                                    # [n, per, d]
    dots = np.einsum("nd,nkd->nk", x64, xg)
    d2p = sq64[:, None] + sq64[partners] - 2.0 * dots
    dpos = np.sqrt(np.clip(d2p, 1e-12, None))
    self_mask = partners == np.arange(n)[:, None]
    dpos_off = np.where(self_mask, np.inf, dpos)

    min_pos = dpos_off.min(axis=1)
    thresh32 = (min_pos + np.float64(MARGIN)).astype(np.float32)
    th64 = thresh32.astype(np.float64)

    dpos_fin = np.where(self_mask, 0.0, dpos)
    sumd_sub = dpos_fin.sum(axis=1)                                   # sum same-class offdiag
    sel_sub = (~self_mask) & (dpos < th64[:, None])
    c_sub = sel_sub.sum(axis=1).astype(np.float64)
    s_sub = np.where(sel_sub, th64[:, None] - dpos, 0.0).sum(axis=1)  # relu correction

    pos_cnt_total = float(n * (per - 1))
    neg_per_row = float(n - per)
    return dict(sq64=sq64, min_pos=min_pos, thresh32=thresh32,
                sumd_sub=sumd_sub, c_sub=c_sub, s_sub=s_sub,
                sum_pos_total=dpos_fin.sum(), pos_cnt_total=pos_cnt_total,
                neg_per_row=neg_per_row)


def _build_inmaps(x, pro):
    sq32 = pro["sq64"].astype(np.float32)
    sq_hi, sq_lo = _trunc_hi(sq32)
    xT = np.ascontiguousarray(x.T)                        # [512, 8192] fp32
    xm2T = (-2.0 * xT).astype(np.float32)

    diagv = np.zeros((128, 4 * 512), np.float32)
    for a in range(4):
        diagv[np.arange(128), a * 512 + a * 128 + np.arange(128)] = BIG
    ident = np.eye(128, dtype=np.float32)
    ones2 = np.ones((2, ROWS_PER_CORE), np.float32)

    in_maps = []
    for c in range(N_CORES):
        shift = -c * ROWS_PER_CORE
        rows = slice(c * ROWS_PER_CORE, (c + 1) * ROWS_PER_CORE)
        sq_own = sq32[rows].reshape(N_RT, 128).T.copy()   # [p, rt]
        th_own = pro["thresh32"][rows].reshape(N_RT, 128).T.copy()
        in_maps.append({
            "xm2T": np.roll(xm2T, shift, axis=1),
            "sqhl": np.roll(np.stack([sq_hi, sq_lo]), shift, axis=1),
            "xTown": np.ascontiguousarray(xT[:, rows]),
            "ones2": ones2,
            "diagv": diagv,
            "ident": ident,
            "sqown": np.ascontiguousarray(sq_own),
            "throw": np.ascontiguousarray(th_own),
        })
    return in_maps


def _assemble(results, pro):
    n = N_ROWS
    sum_raw = np.empty(n)
    cnt_raw = np.empty(n)
    rel_raw = np.empty(n)
    for c in range(N_CORES):
        r = results[c]
        # osum [128, rt*16+ct]; row = c*1024 + rt*128 + p
        s = r["osum"].astype(np.float64).reshape(128, N_RT, N_CT).sum(axis=2)   # [p, rt]
        cn = r["ocnt"].astype(np.float64).reshape(128, N_RT, N_GROUPS).sum(axis=2)
        rl = r["orel"].astype(np.float64).reshape(128, N_RT, N_GROUPS).sum(axis=2)
        rows = slice(c * ROWS_PER_CORE, (c + 1) * ROWS_PER_CORE)
        sum_raw[rows] = s.T.reshape(-1)
        cnt_raw[rows] = cn.T.reshape(-1)
        rel_raw[rows] = rl.T.reshape(-1)

    th64 = pro["thresh32"].astype(np.float64)
    sum_neg = sum_raw - BIG_SQRT - pro["sumd_sub"]
    cnt_neg = cnt_raw - pro["c_sub"]
    s_neg = rel_raw - pro["s_sub"]
    sum_sel = cnt_neg * th64 - s_neg

    valid = cnt_neg > 0.5
    mean_neg = sum_sel / np.maximum(cnt_neg, 1.0)
    row_loss = np.where(valid, pro["min_pos"] - mean_neg + np.float64(MARGIN), 0.0)
    loss = row_loss.sum() / n
    prec = 1.0 - valid.sum() / n
    pos_d = pro["sum_pos_total"] / pro["pos_cnt_total"]
    neg_d = sum_neg.sum() / (n * pro["neg_per_row"])
    return (np.float32(loss), np.float32(prec), np.float32(pos_d), np.float32(neg_d))


def kernel(inputs: np.ndarray, targets: np.ndarray):
    x = np.asarray(inputs, dtype=np.float32)
    t = np.asarray(targets).astype(np.int64)
    pro = _host_prologue(x, t)
    in_maps = _build_inmaps(x, pro)
    run, *_ = _get_runner()
    results = run(in_maps)
    return _assemble(results, pro)


if __name__ == "__main__":
    np.random.seed(1)
    x = np.random.randn(N_ROWS, DIM).astype(np.float32)
    t = (np.arange(N_ROWS) % 1024).astype(np.int32)
    out = kernel(inputs=x, targets=t)
    print("kernel out:", out)
